# revision 3
# baseline (speedup 1.0000x reference)
"""Trainium2 Bass kernel v2 for nn_EncodingNet (FastGTN-style GNN).

Structure (operator form; never materializes dense mats products):
  E_t = densify(edge_index[t], edge_value[t]) as fp16, scattered twice:
    E0: row-sharded   (core k owns rows j with (j%16)//2 == k)
    E1: column-sharded (core k owns cols c with (c%16)//2 == k)
  Each operator application  sum_c mats1[c] @ f(mats0[c] @ V)  becomes a
  LOCAL pair of GEMM passes: pass-0 contracts E0 rows (full RHS needed),
  pass-1 contracts E1 columns against pass-0's local output, yielding a
  full-height PARTIAL that a ReduceScatter sums. Per-type filt coefficients
  are folded into scaled fp16 RHS copies so E_t feeds the PE directly.

  Rounds:  A0/A1 -> RS1 -> local Hc/W1/dinv -> AG1 ->
           C0/D1 -> RS2 -> local h/W2       -> AG2 ->
           E0/F1 -> RS3 -> local log_softmax + per-core target head.
  Host merges per-core partial y (pure row selection).
"""

import sys
import types

import numpy as np

if "antenv.axon_hooks" not in sys.modules:
    _m = types.ModuleType("antenv.axon_hooks")
    _m.get_axon_ntff_profile_hook = lambda: None
    sys.modules["antenv.axon_hooks"] = _m

import concourse.bass as bass
import concourse.bacc as bacc
import concourse.tile as tile
from concourse import mybir

# ---------------------------------------------------------------------------
N = 2048
C = 2
T = 3
L = 2
E = 65536
W_IN = 256
W_OUT = 64
NCLS = 16
NTGT = 512
BETA = 0.5

NCORES = 8
NS = N // NCORES
P = 128
KC = 16
NEL = 1024
NCH = 12                 # scatter chunks per E-set (T*2*2048 / 1024)
EFREE = T * 2 * 2048     # 12288 free elems per E-set tile

f32 = mybir.dt.float32
f16 = mybir.dt.float16
i16 = mybir.dt.int16
i32 = mybir.dt.int32
AF = mybir.ActivationFunctionType
OP = mybir.AluOpType

# misc pack offsets (columns in the [128, MISC_W] fp32 misc tensor)
_MO_CONV = 0          # [12]
_MO_B1 = 12           # [16]
_MO_B2 = 28           # [64]
_MO_LB = 92           # [16]
_MO_ID = 108          # [128]
_MO_GW1 = 236         # [16]  (partitions 0..63)
_MO_GW2 = 252         # [64]  (partitions 0..15)
_MO_LW = 316          # [16]  (partitions 0..63)
_MO_TIDX = 332        # [1]   (int32 bits: local row of target slot)
MISC_W = 334


# ---------------------------------------------------------------------------
# Host-side packing (indexing / bucketing only; no value arithmetic).
# ---------------------------------------------------------------------------
def _bucket_pack(rank, p_of, free, v_all):
    """Generic per-core scatter packing for one E-set.

    Returns (scat_idx [8,P,NCH*NI] i16, plane0 [8,P,NCH*NI] f32,
             dup [8,max(M-1,1),P,NCH*max(DUPW,1)] f32, NI, M, DUPW)."""
    ch_of = free // NEL
    pos_of = free % NEL
    bucket = (rank * P + p_of) * NCH + ch_of
    cell = bucket * NEL + pos_of

    order = np.argsort(cell, kind="stable")
    cell_s = cell[order]
    v_s = v_all[order]
    ucell, first_idx, counts = np.unique(cell_s, return_index=True,
                                         return_counts=True)
    occ = np.arange(len(cell_s)) - np.repeat(first_idx, counts)
    M = int(counts.max())
    ubucket = ucell // NEL
    upos = ucell % NEL
    # duplicated cells first within each bucket -> dup slots < DUPW
    order2 = np.lexsort((np.arange(len(ucell)), counts == 1, ubucket))
    inv2 = np.empty_like(order2)
    inv2[order2] = np.arange(len(order2))
    ub_sorted = ubucket[order2]
    ub_uniq, ub_fidx, ub_counts = np.unique(ub_sorted, return_index=True,
                                            return_counts=True)
    slot_sorted = np.arange(len(ucell)) - np.repeat(ub_fidx, ub_counts)
    slot = slot_sorted[inv2]
    ndup = np.zeros(len(ub_uniq), np.int64)
    isdup_sorted = (counts[order2] >= 2)
    np.add.at(ndup, np.searchsorted(ub_uniq, ub_sorted), isdup_sorted)
    DUPW = int(ndup.max()) if M > 1 else 0
    max_cnt = int(ub_counts.max())
    NI = max_cnt + (max_cnt & 1)

    scat_idx = np.full((NCORES, P, NCH * NI), -1, np.int16)
    plane0 = np.zeros((NCORES, P, NCH * NI), np.float32)
    dup = np.zeros((NCORES, max(M - 1, 1), P, NCH * max(DUPW, 1)),
                   np.float32)
    uk = ubucket // (P * NCH)
    up = (ubucket // NCH) % P
    uch = ubucket % NCH
    scat_idx[uk, up, uch * NI + slot] = upos.astype(np.int16)
    cell_row = np.searchsorted(ucell, cell_s)
    kk = uk[cell_row]
    pp2 = up[cell_row]
    cc = uch[cell_row]
    ss = slot[cell_row]
    m0 = occ == 0
    plane0[kk[m0], pp2[m0], cc[m0] * NI + ss[m0]] = v_s[m0]
    md = ~m0
    if md.any():
        assert (ss[md] < DUPW).all()
        dup[kk[md], occ[md] - 1, pp2[md], cc[md] * DUPW + ss[md]] = v_s[md]
    return scat_idx, plane0, dup, NI, M, DUPW


def _fold(a):  # [256, F] -> [128, 2*F]  (feat = kf*128 + p)
    fdim = a.shape[1]
    return np.ascontiguousarray(
        a.reshape(2, P, fdim).transpose(1, 0, 2).reshape(P, 2 * fdim))


def _prep_inputs(X, edge_value, conv_w, Ws, gcn_w1, gcn_b1, gcn_w2, gcn_b2,
                 lin_w, lin_b, edge_index, target_x):
    X = np.asarray(X, np.float32)
    edge_value = np.asarray(edge_value, np.float32)
    conv_w = np.asarray(conv_w, np.float32)
    Ws = np.asarray(Ws, np.float32)
    gcn_w1 = np.asarray(gcn_w1, np.float32)
    gcn_b1 = np.asarray(gcn_b1, np.float32)
    gcn_w2 = np.asarray(gcn_w2, np.float32)
    gcn_b2 = np.asarray(gcn_b2, np.float32)
    lin_w = np.asarray(lin_w, np.float32)
    lin_b = np.asarray(lin_b, np.float32)
    ei = np.asarray(edge_index, np.int64)
    tx = np.asarray(target_x, np.int64)

    # xT_perm[:, kc*128 + p] = X.T[:, p*16 + kc]
    kk, pp = np.meshgrid(np.arange(KC), np.arange(P), indexing="ij")
    pos_node = (pp * KC + kk).reshape(-1)
    xT_perm = np.ascontiguousarray(X[pos_node].T)          # [256, 2048]
    ws_cat = np.concatenate([Ws[0], Ws[1]], axis=1)        # [256, 128]

    misc = np.zeros((P, MISC_W), np.float32)
    misc[:, _MO_CONV:_MO_CONV + 12] = conv_w.reshape(1, -1)
    misc[:, _MO_B1:_MO_B1 + 16] = gcn_b1.reshape(1, -1)
    misc[:, _MO_B2:_MO_B2 + 64] = gcn_b2.reshape(1, -1)
    misc[:, _MO_LB:_MO_LB + 16] = lin_b.reshape(1, -1)
    misc[:, _MO_ID:_MO_ID + 128] = np.eye(P, dtype=np.float32)
    misc[:64, _MO_GW1:_MO_GW1 + 16] = gcn_w1
    misc[:16, _MO_GW2:_MO_GW2 + 64] = gcn_w2
    misc[:64, _MO_LW:_MO_LW + 16] = lin_w

    # ---- edge bucketing (both shardings) ----------------------------------
    t_id = np.repeat(np.arange(T, dtype=np.int64), E)
    r_all = ei[:, 0, :].reshape(-1)
    c_all = ei[:, 1, :].reshape(-1)
    v_all = edge_value.reshape(-1)

    # E0 row-shard: lhsT[(t,e,kc)][p, i] = E_t[row=i*16+2k+e, col=p*16+kc]
    rank0 = (r_all % 16) >> 1
    p0 = c_all >> 4
    free0 = t_id * 4096 + (r_all & 1) * 2048 + (c_all % 16) * 128 \
        + (r_all >> 4)
    sidx0, pl0, dup0, NI0, M0, DUPW0 = _bucket_pack(rank0, p0, free0, v_all)

    # E1 col-shard: lhsT[(t,ec,kc)][p, i] = E_t[row=i*16+kc, col=p*16+2k+ec]
    rank1 = (c_all % 16) >> 1
    p1 = c_all >> 4
    free1 = t_id * 4096 + (c_all & 1) * 2048 + (r_all % 16) * 128 \
        + (r_all >> 4)
    sidx1, pl1, dup1, NI1, M1, DUPW1 = _bucket_pack(rank1, p1, free1, v_all)

    # ---- per-core target slots --------------------------------------------
    k_of_t = (tx % 16) >> 1
    row_loc = ((tx >> 4) * 2 + (tx & 1)).astype(np.int32)
    tslots = []   # (positions in y, local rows)
    for k in range(NCORES):
        pos = np.nonzero(k_of_t == k)[0]
        assert len(pos) <= P, f"core {k} owns {len(pos)} targets > {P}"
        rows = np.zeros(P, np.int32)
        rows[:len(pos)] = row_loc[pos]
        tslots.append((pos, rows))

    in_maps = []
    for k in range(NCORES):
        # shard rows ordered (e, p): node = p*16 + 2k + e
        ee, pp2 = np.meshgrid(np.arange(2), np.arange(P), indexing="ij")
        nodes = (pp2 * 16 + 2 * k + ee).reshape(-1)
        xmy = _fold(np.ascontiguousarray(X[nodes].T))      # [128, 512]
        big0 = np.concatenate(
            [_fold(xT_perm), _fold(ws_cat), xmy], axis=1).astype(np.float16)
        mk = misc.copy()
        mk[:, _MO_TIDX] = tslots[k][1].view(np.float32)
        m = {
            "big0": np.ascontiguousarray(big0),
            "misc": mk,
            "sidx0": sidx0[k], "sv0": pl0[k],
            "sdup0": np.ascontiguousarray(
                dup0[k].transpose(1, 0, 2).reshape(P, -1)),
            "sidx1": sidx1[k], "sv1": pl1[k],
            "sdup1": np.ascontiguousarray(
                dup1[k].transpose(1, 0, 2).reshape(P, -1)),
        }
        in_maps.append(m)
    meta = (NI0, M0, DUPW0, NI1, M1, DUPW1)
    return in_maps, meta, tslots


# ---------------------------------------------------------------------------
# Device kernel.
# ---------------------------------------------------------------------------
class _StageStop(Exception):
    pass


def build_kernel(meta, reps=1, stop_after=None, skip_coll=False):
    NI0, M0, DUPW0, NI1, M1, DUPW1 = meta
    nc = bacc.Bacc("TRN2", target_bir_lowering=False, debug=False,
                   num_devices=NCORES)
    F0 = NCH * NI0
    F1 = NCH * NI1
    XT_OFF = 0
    WS_OFF = 2 * N
    XMY_OFF = 2 * N + 2 * C * W_OUT
    BIG0_W = XMY_OFF + 2 * NS

    big0_d = nc.dram_tensor("big0", [P, BIG0_W], f16, kind="ExternalInput")
    misc_d = nc.dram_tensor("misc", [P, MISC_W], f32, kind="ExternalInput")
    sidx0_d = nc.dram_tensor("sidx0", [P, F0], i16, kind="ExternalInput")
    sv0_d = nc.dram_tensor("sv0", [P, F0], f32, kind="ExternalInput")
    sd0_d = nc.dram_tensor("sdup0", [P, max(M0 - 1, 1) * NCH *
                                     max(DUPW0, 1)], f32,
                           kind="ExternalInput")
    sidx1_d = nc.dram_tensor("sidx1", [P, F1], i16, kind="ExternalInput")
    sv1_d = nc.dram_tensor("sv1", [P, F1], f32, kind="ExternalInput")
    sd1_d = nc.dram_tensor("sdup1", [P, max(M1 - 1, 1) * NCH *
                                     max(DUPW1, 1)], f32,
                           kind="ExternalInput")
    y_d = nc.dram_tensor("y", [NS, NCLS], f32, kind="ExternalOutput")

    ccds = []
    for r in range(reps):
        cc = {
            "rs1i": nc.dram_tensor(f"rs1i{r}", [N, 130], f32),
            "rs1o": nc.dram_tensor(f"rs1o{r}", [NS, 130], f32),
            "ag1i": nc.dram_tensor(f"ag1i{r}", [NS, 17], f32),
            "ag1o": nc.dram_tensor(f"ag1o{r}", [N, 17], f32,
                                   addr_space="Shared"),
            "rs2i": nc.dram_tensor(f"rs2i{r}", [N, 16], f32),
            "rs2o": nc.dram_tensor(f"rs2o{r}", [NS, 16], f32),
            "ag2i": nc.dram_tensor(f"ag2i{r}", [NS, W_OUT], f32),
            "ag2o": nc.dram_tensor(f"ag2o{r}", [N, W_OUT], f32,
                                   addr_space="Shared"),
            "rs3i": nc.dram_tensor(f"rs3i{r}", [N, W_OUT], f32),
            "rs3o": nc.dram_tensor(f"rs3o{r}", [NS, W_OUT], f32),
        }
        ccds.append(cc)
    rg = [list(range(NCORES))]

    # scatter chunk order (see free-layout): E0 e=0 chunks first, then e=1;
    # E1 low-kc chunks first.
    E0_ORDER = [0, 1, 4, 5, 8, 9, 2, 3, 6, 7, 10, 11]
    E1_ORDER = [0, 2, 4, 6, 8, 10, 1, 3, 5, 7, 9, 11]

    with tile.TileContext(nc) as tc:
        import contextlib
        ctx = contextlib.ExitStack()
        with ctx:
            pool = ctx.enter_context(tc.tile_pool(name="main", bufs=1))
            ppool = ctx.enter_context(
                tc.tile_pool(name="pass_psum", bufs=4, space="PSUM"))
            apool = ctx.enter_context(
                tc.tile_pool(name="aux_psum", bufs=3, space="PSUM"))

            # ---------------- input loads ----------------
            misc = pool.tile([P, MISC_W], f32, tag="misc")
            nc.sync.dma_start(misc[:], misc_d[:])
            sv0_sb = pool.tile([P, F0], f32, tag="sv0")
            nc.sync.dma_start(sv0_sb[:], sv0_d[:])
            sidx0_sb = pool.tile([P, F0], i16, tag="sidx0")
            nc.sync.dma_start(sidx0_sb[:], sidx0_d[:])
            sd0_sb = pool.tile([P, sd0_d.shape[1]], f32, tag="sd0")
            nc.sync.dma_start(sd0_sb[:], sd0_d[:])
            sv1_sb = pool.tile([P, F1], f32, tag="sv1")
            nc.sync.dma_start(sv1_sb[:], sv1_d[:])
            sidx1_sb = pool.tile([P, F1], i16, tag="sidx1")
            nc.sync.dma_start(sidx1_sb[:], sidx1_d[:])
            sd1_sb = pool.tile([P, sd1_d.shape[1]], f32, tag="sd1")
            nc.sync.dma_start(sd1_sb[:], sd1_d[:])
            big0 = pool.tile([P, BIG0_W], f16, tag="big0")
            nc.sync.dma_start(big0[:], big0_d[:])

            ident = misc[:, _MO_ID:_MO_ID + 128]
            b1_ap = misc[:, _MO_B1:_MO_B1 + 16]
            b2_ap = misc[:, _MO_B2:_MO_B2 + 64]
            lb_ap = misc[:, _MO_LB:_MO_LB + 16]
            gw1_ap = misc[0:64, _MO_GW1:_MO_GW1 + 16]
            gw2_ap = misc[0:16, _MO_GW2:_MO_GW2 + 64]
            lw_ap = misc[0:64, _MO_LW:_MO_LW + 16]
            tidx_ap = misc[:, _MO_TIDX:_MO_TIDX + 1].bitcast(i32)

            # persistent tiles
            e0sb = pool.tile([P, EFREE], f16, tag="e0sb")
            e1sb = pool.tile([P, EFREE], f16, tag="e1sb")
            rhs_a = pool.tile([P, KC, 130], f16, tag="rhs_a")
            nc.vector.memset(rhs_a[:], 1.0)

            prev_y = None
            stage_state = {}

            def _stage(name, tile_ref):
                stage_state["last"] = tile_ref
                if stop_after == name:
                    raise _StageStop()

            for rep in range(reps):
                try:
                    # ---------- filt = softmax(conv_w) ----------
                    ex = pool.tile([P, L * C * T], f32, tag="ex")
                    nc.scalar.activation(ex[:],
                                         misc[:, _MO_CONV:_MO_CONV + 12],
                                         AF.Exp)
                    sums = pool.tile([P, L * C], f32, tag="sums")
                    nc.vector.tensor_reduce(
                        sums[:], ex[:].rearrange("p (g t) -> p g t", t=T),
                        axis=mybir.AxisListType.X, op=OP.add)
                    rec = pool.tile([P, L * C], f32, tag="rec")
                    nc.vector.reciprocal(rec[:], sums[:])
                    filt = pool.tile([P, L * C * T], f32, tag="filt")
                    for g in range(L * C):
                        nc.vector.tensor_scalar_mul(
                            filt[:, g * T:(g + 1) * T],
                            ex[:, g * T:(g + 1) * T], rec[:, g:g + 1])

                    def fs(l, c, t):
                        q = (l * C + c) * T + t
                        return filt[:, q:q + 1]

                    # ---------- dup-sum (rep 0) + serialization ----------
                    if rep == 0:
                        for (sv, sd, M_, DUPW_, NI_) in (
                                (sv0_sb, sd0_sb, M0, DUPW0, NI0),
                                (sv1_sb, sd1_sb, M1, DUPW1, NI1)):
                            if M_ > 1 and DUPW_ > 0:
                                vv = sv[:].rearrange("p (c s) -> p c s",
                                                     c=NCH)
                                dd = sd[:].rearrange("p (m c s) -> p m c s",
                                                     m=M_ - 1, c=NCH)
                                for m in range(M_ - 1):
                                    nc.vector.tensor_add(
                                        vv[:, :, 0:DUPW_],
                                        vv[:, :, 0:DUPW_], dd[:, m])
                    if prev_y is not None:
                        jz = pool.tile([P, 1], f32, tag="jz")
                        nc.vector.tensor_scalar_mul(jz[:], prev_y, 0.0)
                        nc.vector.tensor_scalar_add(sv0_sb[:, 0:1],
                                                    sv0_sb[:, 0:1], jz[:, :])
                        nc.vector.tensor_scalar_add(sv1_sb[:, 0:1],
                                                    sv1_sb[:, 0:1], jz[:, :])
                        nc.vector.tensor_scalar_add(big0[:, 0:1],
                                                    big0[:, 0:1], jz[:, :])

                    # ---------- quantize to fp16 + scatter ----------
                    vq0 = pool.tile([P, F0], f16, tag="vq0")
                    nc.scalar.activation(vq0[:], sv0_sb[:], AF.Copy)
                    vq1 = pool.tile([P, F1], f16, tag="vq1")
                    nc.scalar.activation(vq1[:], sv1_sb[:], AF.Copy)
                    for ch in E0_ORDER:
                        nc.gpsimd.local_scatter(
                            out_ap=e0sb[:, ch * NEL:(ch + 1) * NEL],
                            data_ap=vq0[:, ch * NI0:(ch + 1) * NI0],
                            idxs_ap=sidx0_sb[:, ch * NI0:(ch + 1) * NI0],
                            channels=P, num_elems=NEL, num_idxs=NI0)
                    for ch in E1_ORDER:
                        nc.gpsimd.local_scatter(
                            out_ap=e1sb[:, ch * NEL:(ch + 1) * NEL],
                            data_ap=vq1[:, ch * NI1:(ch + 1) * NI1],
                            idxs_ap=sidx1_sb[:, ch * NI1:(ch + 1) * NI1],
                            channels=P, num_elems=NEL, num_idxs=NI1)
                    _stage("ebuild", e1sb[:, 0:1])

                    def e0chunk(t, e, kc):
                        o = t * 4096 + e * 2048 + kc * 128
                        return e0sb[:, o:o + 128]

                    def e1chunk(t, ec, kc):
                        o = t * 4096 + ec * 2048 + kc * 128
                        return e1sb[:, o:o + 128]

                    # ---------- rhs_a = [X_|1 | X_|1] fp16 ----------
                    for kc in range(KC):
                        ps = apool.tile([P, C * W_OUT], f32, space="PSUM",
                                        tag="aux")
                        for a in range(2):
                            nc.tensor.matmul(
                                ps[:],
                                big0[:, XT_OFF + a * N + kc * P:
                                     XT_OFF + a * N + (kc + 1) * P],
                                big0[:, WS_OFF + a * C * W_OUT:
                                     WS_OFF + (a + 1) * C * W_OUT],
                                start=(a == 0), stop=(a == 1))
                        nc.scalar.activation(
                            rhs_a[:, kc, :].rearrange(
                                "p (b q) -> p b q", q=65)[:, :, 0:64],
                            ps[:].rearrange("p (b q) -> p b q", q=64),
                            AF.Copy)

                    # X_sh for the Hc tail: [128, 2, 128] fp32
                    xsh = pool.tile([P, 2, C * W_OUT], f32, tag="xsh")
                    for e in range(2):
                        ps = apool.tile([P, C * W_OUT], f32, space="PSUM",
                                        tag="aux")
                        for a in range(2):
                            nc.tensor.matmul(
                                ps[:],
                                big0[:, XMY_OFF + a * 2 * P + e * P:
                                     XMY_OFF + a * 2 * P + (e + 1) * P],
                                big0[:, WS_OFF + a * C * W_OUT:
                                     WS_OFF + (a + 1) * C * W_OUT],
                                start=(a == 0), stop=(a == 1))
                        nc.vector.tensor_copy(xsh[:, e, :], ps[:])

                    # ---------- xs[t] = per-type scaled rhs_a ----------
                    xs = [pool.tile([P, KC, 130], f16, tag=f"xs{t}",
                                    name=f"xs{t}")
                          for t in range(T)]
                    for t in range(T):
                        for c in range(C):
                            nc.vector.tensor_scalar_mul(
                                xs[t][:, :, c * 65:(c + 1) * 65],
                                rhs_a[:, :, c * 65:(c + 1) * 65],
                                fs(0, c, t))

                    # ================ PASS A0 ================
                    s0 = pool.tile([P, 2, 130], f16, tag="s0")
                    for e in range(2):
                        ps = ppool.tile([P, 130], f32, space="PSUM",
                                        tag="ep")
                        first = True
                        for t in range(T):
                            for kc in range(KC):
                                nc.tensor.matmul(
                                    ps[:], e0chunk(t, e, kc),
                                    xs[t][:, kc, :], start=first,
                                    stop=(t == T - 1 and kc == KC - 1))
                                first = False
                        nc.scalar.activation(s0[:, e, :], ps[:], AF.Copy)

                    # ss[t] = per-type scaled s0
                    ss = [pool.tile([P, 2, 130], f16, tag=f"ss{t}",
                                    name=f"ss{t}")
                          for t in range(T)]
                    for t in range(T):
                        for c in range(C):
                            nc.vector.tensor_scalar_mul(
                                ss[t][:, :, c * 65:(c + 1) * 65],
                                s0[:, :, c * 65:(c + 1) * 65], fs(1, c, t))

                    # ================ PASS A1 ================
                    stg1 = pool.tile([P, KC, 130], f32, tag="stg1")
                    for kc in range(KC):
                        ps = ppool.tile([P, 130], f32, space="PSUM",
                                        tag="ep")
                        first = True
                        for t in range(T):
                            for ec in range(2):
                                nc.tensor.matmul(
                                    ps[:], e1chunk(t, ec, kc),
                                    ss[t][:, ec, :], start=first,
                                    stop=(t == T - 1 and ec == 1))
                                first = False
                        nc.scalar.activation(stg1[:, kc, :], ps[:], AF.Copy)
                    _stage("passA", stg1[:, 0, 0:1])

                    cc = ccds[rep]

                    def rs_round(name, stage_sb, d, cin, cout):
                        cv = cin[:].rearrange("(g p e) d -> g p e d", p=P,
                                              e=2)
                        for g in range(NCORES):
                            nc.sync.dma_start(cv[g],
                                              stage_sb[:, 2 * g:2 * g + 2,
                                                       :])
                        if not skip_coll:
                            nc.gpsimd.collective_compute(
                                "ReduceScatter", OP.add, replica_groups=rg,
                                ins=[cin[:]], outs=[cout[:]])
                        rsh = pool.tile([P, 2, d], f32, tag=f"rsh_{name}")
                        nc.sync.dma_start(
                            rsh[:],
                            cout[:].rearrange("(p e) d -> p e d", e=2))
                        return rsh

                    rsh1 = rs_round("1", stg1, 130, cc["rs1i"], cc["rs1o"])
                    _stage("rs1", rsh1[:, 0, 0:1])

                    # ---------- round-1 tail: deg/dinv, Hc, W1 ----------
                    dg = pool.tile([P, 2], f32, tag="deg")
                    nc.vector.tensor_add(dg[:], rsh1[:, :, 64],
                                         rsh1[:, :, 129])
                    nc.vector.tensor_scalar_add(dg[:], dg[:], 1.0)
                    sq = pool.tile([P, 2], f32, tag="sq")
                    nc.scalar.activation(sq[:], dg[:], AF.Sqrt)
                    dinv = pool.tile([P, 2], f32, tag="dinv")
                    nc.vector.reciprocal(dinv[:], sq[:])

                    hcT = pool.tile([W_OUT, 2 * P], f32, tag="hcT")
                    for e in range(2):
                        t1 = pool.tile([P, 2, W_OUT], f32, tag="hct1",
                                       bufs=2)
                        nc.vector.tensor_add(
                            t1[:],
                            xsh[:, e, :].rearrange("p (b q) -> p b q",
                                                   q=64),
                            rsh1[:, e, :].rearrange(
                                "p (b q) -> p b q", q=65)[:, :, 0:64])
                        r1 = pool.tile([P, 2, W_OUT], f32, tag="hcr1",
                                       bufs=2)
                        nc.scalar.activation(r1[:], t1[:], AF.Relu,
                                             scale=BETA)
                        hc_e = pool.tile([P, W_OUT], f32, tag="hc_e",
                                         bufs=2)
                        nc.vector.tensor_add(hc_e[:], r1[:, 0, :],
                                             r1[:, 1, :])
                        nc.vector.tensor_scalar_mul(hc_e[:], hc_e[:], 0.5)
                        tp = apool.tile([P, P], f32, space="PSUM",
                                        tag="aux")
                        nc.tensor.transpose(tp[:W_OUT, :], hc_e[:], ident)
                        nc.vector.tensor_copy(hcT[:, e * P:(e + 1) * P],
                                              tp[:W_OUT, :])
                    w1d = pool.tile([P, 2, 17], f32, tag="w1d")
                    for e in range(2):
                        psz = apool.tile([P, 16], f32, space="PSUM",
                                         tag="aux")
                        nc.tensor.matmul(psz[:], hcT[:, e * P:(e + 1) * P],
                                         gw1_ap, start=True, stop=True)
                        nc.vector.tensor_scalar_mul(w1d[:, e, 0:16], psz[:],
                                                    dinv[:, e:e + 1])
                    nc.vector.tensor_copy(w1d[:, :, 16], dinv[:, :])

                    def ag_round(name, shard_sb, d, cin, cout):
                        nc.sync.dma_start(
                            cin[:].rearrange("(p e) d -> p e d", e=2),
                            shard_sb[:])
                        if not skip_coll:
                            nc.gpsimd.collective_compute(
                                "AllGather", OP.bypass, replica_groups=rg,
                                ins=[cin[:]], outs=[cout[:]])
                        rhs = pool.tile([P, KC, d], f32, tag=f"rhs_{name}")
                        cov = cout[:].rearrange("(g p e) d -> g p e d",
                                                p=P, e=2)
                        for g in range(NCORES):
                            nc.sync.dma_start(rhs[:, 2 * g:2 * g + 2, :],
                                              cov[g])
                        return rhs

                    rhs_c = ag_round("c", w1d, 17, cc["ag1i"], cc["ag1o"])
                    _stage("ag1", rhs_c[:, 0, 0:1])

                    # ---------- ws1[t]; PASS C0 ----------
                    ws1 = [pool.tile([P, KC, 2 * 16], f16, tag=f"ws1{t}",
                                     name=f"ws1{t}")
                           for t in range(T)]
                    for t in range(T):
                        for c in range(C):
                            if (t * C + c) % 2 == 0:
                                nc.vector.tensor_scalar_mul(
                                    ws1[t][:, :, c * 16:(c + 1) * 16],
                                    rhs_c[:, :, 0:16], fs(0, c, t))
                            else:
                                nc.scalar.activation(
                                    ws1[t][:, :, c * 16:(c + 1) * 16],
                                    rhs_c[:, :, 0:16], AF.Copy,
                                    scale=fs(0, c, t))
                    s1 = pool.tile([P, 2, 2 * 16], f16, tag="s1")
                    for e in range(2):
                        ps = ppool.tile([P, 2 * 16], f32, space="PSUM",
                                        tag="ep")
                        first = True
                        for t in range(T):
                            for kc in range(KC):
                                nc.tensor.matmul(
                                    ps[:], e0chunk(t, e, kc),
                                    ws1[t][:, kc, :], start=first,
                                    stop=(t == T - 1 and kc == KC - 1))
                                first = False
                        nc.scalar.activation(s1[:, e, :], ps[:], AF.Copy)

                    # rhsD[t] = fs(1,0,t)*s1_c0 + fs(1,1,t)*s1_c1
                    rhsD = [pool.tile([P, 2, 16], f16, tag=f"rhsD{t}",
                                      name=f"rhsD{t}")
                            for t in range(T)]
                    for t in range(T):
                        tmp = pool.tile([P, 2, 16], f16, tag="tmpD", bufs=2)
                        nc.vector.tensor_scalar_mul(
                            tmp[:], s1[:, :, 0:16], fs(1, 0, t))
                        nc.vector.scalar_tensor_tensor(
                            out=rhsD[t][:], in0=s1[:, :, 16:32],
                            scalar=fs(1, 1, t), in1=tmp[:],
                            op0=OP.mult, op1=OP.add)

                    # ================ PASS D1 ================
                    stgD = pool.tile([P, KC, 16], f32, tag="stgD")
                    for kc in range(KC):
                        ps = ppool.tile([P, 16], f32, space="PSUM",
                                        tag="ep")
                        first = True
                        for t in range(T):
                            for ec in range(2):
                                nc.tensor.matmul(
                                    ps[:], e1chunk(t, ec, kc),
                                    rhsD[t][:, ec, :], start=first,
                                    stop=(t == T - 1 and ec == 1))
                                first = False
                        nc.scalar.activation(stgD[:, kc, :], ps[:], AF.Copy)
                    _stage("passD", stgD[:, 0, 0:1])

                    rsh2 = rs_round("2", stgD, 16, cc["rs2i"], cc["rs2o"])
                    _stage("rs2", rsh2[:, 0, 0:1])

                    # ---------- round-2 tail: h, W2 ----------
                    hT = pool.tile([16, 2 * P], f32, tag="hT")
                    w2sh = pool.tile([P, 2, W_OUT], f32, tag="w2sh")
                    for e in range(2):
                        aw = pool.tile([P, 16], f32, tag="aw1", bufs=2)
                        nc.vector.tensor_add(aw[:], rsh2[:, e, :],
                                             w1d[:, e, 0:16])
                        nc.vector.scalar_tensor_tensor(
                            out=aw[:], in0=aw[:], scalar=dinv[:, e:e + 1],
                            in1=b1_ap, op0=OP.mult, op1=OP.add)
                        h_e = pool.tile([P, 16], f32, tag="h_e", bufs=2)
                        nc.vector.tensor_scalar_max(h_e[:], aw[:], 0.0)
                        tp = apool.tile([P, P], f32, space="PSUM",
                                        tag="aux")
                        nc.tensor.transpose(tp[:16, :], h_e[:], ident)
                        nc.vector.tensor_copy(hT[:, e * P:(e + 1) * P],
                                              tp[:16, :])
                    for e in range(2):
                        psz = apool.tile([P, W_OUT], f32, space="PSUM",
                                         tag="aux")
                        nc.tensor.matmul(psz[:], hT[:, e * P:(e + 1) * P],
                                         gw2_ap, start=True, stop=True)
                        nc.vector.tensor_scalar_mul(w2sh[:, e, :], psz[:],
                                                    dinv[:, e:e + 1])

                    rhs_e = ag_round("e", w2sh, W_OUT, cc["ag2i"],
                                     cc["ag2o"])
                    _stage("ag2", rhs_e[:, 0, 0:1])

                    # ---------- ws2[t]; PASS E0 ----------
                    ws2 = [pool.tile([P, KC, 2 * W_OUT], f16,
                                     tag=f"ws2{t}", name=f"ws2{t}")
                           for t in range(T)]
                    for t in range(T):
                        for c in range(C):
                            if (t * C + c) % 2 == 0:
                                nc.vector.tensor_scalar_mul(
                                    ws2[t][:, :, c * 64:(c + 1) * 64],
                                    rhs_e[:], fs(0, c, t))
                            else:
                                nc.scalar.activation(
                                    ws2[t][:, :, c * 64:(c + 1) * 64],
                                    rhs_e[:], AF.Copy, scale=fs(0, c, t))
                    s2 = pool.tile([P, 2, 2 * W_OUT], f16, tag="s2")
                    for e in range(2):
                        ps = ppool.tile([P, 2 * W_OUT], f32, space="PSUM",
                                        tag="ep")
                        first = True
                        for t in range(T):
                            for kc in range(KC):
                                nc.tensor.matmul(
                                    ps[:], e0chunk(t, e, kc),
                                    ws2[t][:, kc, :], start=first,
                                    stop=(t == T - 1 and kc == KC - 1))
                                first = False
                        nc.scalar.activation(s2[:, e, :], ps[:], AF.Copy)

                    rhsF = [pool.tile([P, 2, W_OUT], f16, tag=f"rhsF{t}",
                                      name=f"rhsF{t}")
                            for t in range(T)]
                    for t in range(T):
                        tmp = pool.tile([P, 2, W_OUT], f16, tag="tmpF",
                                        bufs=2)
                        nc.vector.tensor_scalar_mul(
                            tmp[:], s2[:, :, 0:W_OUT], fs(1, 0, t))
                        nc.vector.scalar_tensor_tensor(
                            out=rhsF[t][:], in0=s2[:, :, W_OUT:2 * W_OUT],
                            scalar=fs(1, 1, t), in1=tmp[:],
                            op0=OP.mult, op1=OP.add)

                    # ================ PASS F1 ================
                    stgF = pool.tile([P, KC, W_OUT], f32, tag="stgF")
                    for kc in range(KC):
                        ps = ppool.tile([P, W_OUT], f32, space="PSUM",
                                        tag="ep")
                        first = True
                        for t in range(T):
                            for ec in range(2):
                                nc.tensor.matmul(
                                    ps[:], e1chunk(t, ec, kc),
                                    rhsF[t][:, ec, :], start=first,
                                    stop=(t == T - 1 and ec == 1))
                                first = False
                        nc.scalar.activation(stgF[:, kc, :], ps[:], AF.Copy)
                    _stage("passF", stgF[:, 0, 0:1])

                    rsh3 = rs_round("3", stgF, W_OUT, cc["rs3i"],
                                    cc["rs3o"])
                    _stage("rs3", rsh3[:, 0, 0:1])

                    # ---------- round-3 tail: log_softmax + head ----------
                    hls = pool.tile([P, 2, W_OUT], f32, tag="hls")
                    for e in range(2):
                        aw = pool.tile([P, W_OUT], f32, tag="aw2", bufs=2)
                        nc.vector.tensor_add(aw[:], rsh3[:, e, :],
                                             w2sh[:, e, :])
                        nc.vector.scalar_tensor_tensor(
                            out=aw[:], in0=aw[:], scalar=dinv[:, e:e + 1],
                            in1=b2_ap, op0=OP.mult, op1=OP.add)
                        mx = pool.tile([P, 1], f32, tag="mx", bufs=2)
                        nc.vector.tensor_reduce(mx[:], aw[:],
                                                axis=mybir.AxisListType.X,
                                                op=OP.max)
                        nmx = pool.tile([P, 1], f32, tag="nmx", bufs=2)
                        nc.vector.tensor_scalar_mul(nmx[:], mx[:], -1.0)
                        ee = pool.tile([P, W_OUT], f32, tag="ee", bufs=2)
                        nc.scalar.activation(ee[:], aw[:], AF.Exp,
                                             bias=nmx[:, :])
                        ssum = pool.tile([P, 1], f32, tag="ssum", bufs=2)
                        nc.vector.tensor_reduce(ssum[:], ee[:],
                                                axis=mybir.AxisListType.X,
                                                op=OP.add)
                        lns = pool.tile([P, 1], f32, tag="lns", bufs=2)
                        nc.scalar.activation(lns[:], ssum[:], AF.Ln)
                        tot = pool.tile([P, 1], f32, tag="tot", bufs=2)
                        nc.vector.tensor_add(tot[:], mx[:], lns[:])
                        nc.vector.tensor_scalar(out=hls[:, e, :],
                                                in0=aw[:],
                                                scalar1=tot[:, :],
                                                scalar2=None,
                                                op0=OP.subtract)

                    # head on ALL 256 local rows; host selects target rows
                    y_sb = pool.tile([P, 2, NCLS], f32, tag="y_sb")
                    for e in range(2):
                        tp = apool.tile([P, P], f32, space="PSUM",
                                        tag="aux")
                        nc.tensor.transpose(tp[:W_OUT, :], hls[:, e, :],
                                            ident)
                        gT = pool.tile([W_OUT, P], f32, tag="gT", bufs=2)
                        nc.vector.tensor_copy(gT[:], tp[:W_OUT, :])
                        psy = apool.tile([P, NCLS], f32, space="PSUM",
                                         tag="aux")
                        nc.tensor.matmul(psy[:], gT[:], lw_ap, start=True,
                                         stop=True)
                        nc.vector.tensor_add(y_sb[:, e, :], psy[:], lb_ap)
                    nc.sync.dma_start(
                        y_d[:].rearrange("(p e) d -> p e d", e=2),
                        y_sb[:])
                except _StageStop:
                    lt = stage_state["last"]
                    y_sb = pool.tile([P, 2, NCLS], f32, tag="ydummy")
                    nc.vector.memset(y_sb[:], 0.0)
                    nc.vector.tensor_scalar_mul(y_sb[:, 0, 0:1], lt, 0.0)
                    nc.sync.dma_start(
                        y_d[:].rearrange("(p e) d -> p e d", e=2),
                        y_sb[:])
                prev_y = y_sb[:, 0, 0:1]

    nc.compile()
    return nc


# ---------------------------------------------------------------------------
# Execution via PJRT (axon).
# ---------------------------------------------------------------------------
class _Runner:
    def __init__(self, nc, n_cores):
        import jax
        from jax.sharding import Mesh, PartitionSpec
        from jax.experimental.shard_map import shard_map
        from concourse.bass2jax import (
            _bass_exec_p, install_neuronx_cc_hook, partition_id_tensor)

        install_neuronx_cc_hook()
        self.jax = jax
        self._nc = nc
        self.n_cores = n_cores
        partition_name = (
            nc.partition_id_tensor.name if nc.partition_id_tensor else None)
        in_names, out_names, out_avals, zero_outs = [], [], [], []
        for alloc in nc.m.functions[0].allocations:
            if not isinstance(alloc, mybir.MemoryLocationSet):
                continue
            name = alloc.memorylocations[0].name
            if alloc.kind == "ExternalInput":
                if name != partition_name:
                    in_names.append(name)
            elif alloc.kind == "ExternalOutput":
                shape = tuple(alloc.tensor_shape)
                dtype = mybir.dt.np(alloc.dtype)
                out_names.append(name)
                out_avals.append(jax.core.ShapedArray(shape, dtype))
                zero_outs.append(np.zeros(shape, dtype))
        self.n_params = len(in_names)
        self.out_names = out_names
        self.out_avals = out_avals
        self.zero_outs = zero_outs
        n_outs = len(out_avals)
        in_names = in_names + out_names
        if partition_name is not None:
            in_names.append(partition_name)
        self.in_names = in_names

        def _body(*args):
            operands = list(args)
            if partition_name is not None:
                operands.append(partition_id_tensor())
            outs = _bass_exec_p.bind(
                *operands, out_avals=tuple(out_avals),
                in_names=tuple(in_names), out_names=tuple(out_names),
                lowering_input_output_aliases=(),
                sim_require_finite=True, sim_require_nnan=True, nc=nc)
            return tuple(outs)

        devices = jax.devices()[:n_cores]
        mesh = Mesh(np.asarray(devices), ("core",))
        in_specs = (PartitionSpec("core"),) * (self.n_params + n_outs)
        out_specs = (PartitionSpec("core"),) * n_outs
        self._fn = jax.jit(
            shard_map(_body, mesh=mesh, in_specs=in_specs,
                      out_specs=out_specs, check_rep=False),
            donate_argnums=tuple(range(self.n_params,
                                       self.n_params + n_outs)),
            keep_unused=True)

    def concat_inputs(self, in_maps):
        return [
            np.concatenate([np.asarray(m[name]) for m in in_maps], axis=0)
            for name in self.in_names[: self.n_params]
        ]

    def zeros(self):
        return [
            np.zeros((self.n_cores * z.shape[0], *z.shape[1:]), z.dtype)
            for z in self.zero_outs
        ]

    def run(self, in_maps):
        outs = self._fn(*self.concat_inputs(in_maps), *self.zeros())
        return [
            {
                name: np.asarray(outs[i]).reshape(
                    self.n_cores, *self.out_avals[i].shape)[c]
                for i, name in enumerate(self.out_names)
            }
            for c in range(self.n_cores)
        ]


_CACHE = {}


def _get_runner(meta, reps=1, stop_after=None, skip_coll=False):
    key = (meta, reps, stop_after, skip_coll)
    if key not in _CACHE:
        nc = build_kernel(meta, reps=reps, stop_after=stop_after,
                          skip_coll=skip_coll)
        _CACHE[key] = _Runner(nc, NCORES)
    return _CACHE[key]


def kernel(**inputs) -> np.ndarray:
    in_maps, meta, tslots = _prep_inputs(**inputs)
    runner = _get_runner(meta)
    results = runner.run(in_maps)
    y = np.zeros((NTGT, NCLS), np.float32)
    for k in range(NCORES):
        pos, rows = tslots[k]
        y[pos] = results[k]["y"][rows[: len(pos)]]
    return y
